# revision 38
# baseline (speedup 1.0000x reference)
"""FrameDockingScoreModel forward as a Trainium2 Bass kernel (8 NeuronCores).

Contract: kernel(**inputs) takes the FULL unsharded inputs from
setup_inputs() and returns (s_rot, s_tr, t_rot, t_tr), float32 (B, 3) each.

Math: the two chained e3nn tensor products are linearized at build time into
one K=124 -> M=42 matmul over per-edge exponential features:
  s1p*s2q*s3v = exp(coeff*(a1[p]+a2[q]+a3[v]) + bias_pqv),  a_j[i] = d2_j - 2*off_i*d_j
so the whole Gaussian-product feature block is ONE activation over a
transposed argument tile.  Per 16384-edge sub-group, per core:
  A: geometry (edge-major SoA tiles, DVE/Pool/ACT) -> argument tile AR (fp16)
  B: PE-transpose AR chunks -> PSUM, ACT exp (PSUM->SBUF bf16 K-tile),
     bf16 matmul vs folded weight table, escape-cast, HW DMA-transpose back
  C: edge-major assembly of the 12 outputs (bf16) -> DMA out
Stages are software-pipelined (A(s), B(s-1), C(s-2) emission skew) so each
engine's program order interleaves independent sub-groups.
Batch is padded to 8 * S * 128 * C edges and split across the 8 cores.
"""
import numpy as np
import ml_dtypes
from contextlib import ExitStack

import concourse.bacc as bacc
import concourse.tile as tile
from concourse import mybir
from concourse.bass_utils import run_bass_kernel_spmd

NG = 4
MAX_OFF = 5.0
C1_000 = (1.0 / 17.0) ** 0.5
C1_110 = (1.0 / 17.0) ** 0.5 / 3.0 ** 0.5
C1_011 = (3.0 / 9.0) ** 0.5 / 3.0 ** 0.5
C1_101 = C1_011
C1_111 = (3.0 / 9.0) ** 0.5 / 6.0 ** 0.5
C2_000 = (1.0 / 85.0) ** 0.5
C2_110 = (1.0 / 85.0) ** 0.5 / 3.0 ** 0.5
C2_011 = (3.0 / 45.0) ** 0.5 / 3.0 ** 0.5
C2_101 = C2_011
C2_111 = (3.0 / 45.0) ** 0.5 / 6.0 ** 0.5

F32 = mybir.dt.float32
F16 = mybir.dt.float16
BF16 = mybir.dt.bfloat16
AF = mybir.ActivationFunctionType
OP = mybir.AluOpType

OFF = np.linspace(0.0, MAX_OFF, NG).astype(np.float64)
COEFF = -0.5 / (OFF[1] - OFF[0]) ** 2

N_CORES = 8
C_CHUNK = 128            # edges-per-partition per sub-group


def build_weight_tables(w):
    w = {k: np.asarray(v, np.float64) for k, v in w.items()}
    G1 = C2_000 * C1_000 * np.einsum('uvw,pqu->pqvw', w['w2_000'], w['w1_000'])
    G2 = C2_000 * C1_110 * np.einsum('uvw,u->vw', w['w2_000'], w['w1_110'])
    G3 = C2_110 * C1_011 * np.einsum('uw,pu->pw', w['w2_110'], w['w1_011'])
    G4 = C2_110 * C1_101 * np.einsum('uw,qu->qw', w['w2_110'], w['w1_101'])
    G5 = C2_110 * C1_111 * np.einsum('uw,u->w', w['w2_110'], w['w1_111'])
    H1p = C2_011 * C1_000 * np.einsum('u,pqu->pq', w['w2_011'], w['w1_000'])
    h2p = C2_011 * C1_110 * float(w['w2_011'] @ w['w1_110'])
    H3 = C2_101 * C1_011 * np.einsum('uv,pu->pv', w['w2_101'], w['w1_011'])
    H4 = C2_101 * C1_101 * np.einsum('uv,qu->qv', w['w2_101'], w['w1_101'])
    H5 = C2_101 * C1_111 * np.einsum('uv,u->v', w['w2_101'], w['w1_111'])
    E1 = C2_111 * C1_011 * np.einsum('u,pu->p', w['w2_111'], w['w1_011'])
    E2 = C2_111 * C1_101 * np.einsum('u,qu->q', w['w2_111'], w['w1_101'])
    kap = C2_111 * C1_111 * float(w['w2_111'] @ w['w1_111'])

    Wk = np.zeros((124, 64), np.float64)
    r = 0
    Wk[r:r + 64, 0:9] = G1.reshape(64, 9); r += 64       # s123 -> S2p
    Wk[r:r + 16, 36] = H1p.reshape(16); r += 16          # s12 -> alpha
    Wk[r:r + 16, 37] = H3.reshape(16); r += 16           # s13 -> beta
    Wk[r:r + 16, 38] = H4.reshape(16); r += 16           # s23 -> gamma
    Wk[r:r + 4, 18:27] = G3; Wk[r:r + 4, 40] = E1; r += 4    # s1 -> T3, z1
    Wk[r:r + 4, 27:36] = G4; Wk[r:r + 4, 41] = -E2; r += 4   # s2 -> T4, z2m
    Wk[r:r + 4, 9:18] = G2; Wk[r:r + 4, 39] = H5; r += 4     # s3 -> T2, delta'
    assert r == 124

    o2 = OFF ** 2
    p_, q_, v_ = np.meshgrid(np.arange(4), np.arange(4), np.arange(4), indexing='ij')
    pq = np.meshgrid(np.arange(4), np.arange(4), indexing='ij')
    cb = np.concatenate([
        o2[p_.ravel()] + o2[q_.ravel()] + o2[v_.ravel()],
        o2[pq[0].ravel()] + o2[pq[1].ravel()],
        o2[pq[0].ravel()] + o2[pq[1].ravel()],
        o2[pq[0].ravel()] + o2[pq[1].ravel()],
        o2, o2, o2,
        np.zeros(4),
    ]) * COEFF
    cbias = cb.reshape(128, 1).astype(np.float32)
    return (Wk.astype(np.float32), cbias, G5.astype(np.float32),
            float(h2p), float(kap))


def build_nc(S, C, weights, n_cores, reps=1, TB=8, PB=4):
    Wk64, cbias, G5, h2p, kap = build_weight_tables(weights)
    NB = C // PB
    assert NB % TB == 0

    nc = bacc.Bacc("TRN2", target_bir_lowering=False, debug=False,
                   num_devices=n_cores)
    lig = nc.declare_dram_parameter("lig", [S, 128, 9, C], F32, isOutput=False)
    rec = nc.declare_dram_parameter("rec", [S, 128, 9, C], F32, isOutput=False)
    out = nc.declare_dram_parameter("out", [S, 128, 12, C], BF16, isOutput=True)

    wk_d = nc.inline_tensor(Wk64.astype(ml_dtypes.bfloat16), name="wk")
    cb_d = nc.inline_tensor(cbias, name="cbias")
    id_d = nc.inline_tensor(np.eye(128, dtype=np.float16), name="ident16")

    with tile.TileContext(nc) as tc, ExitStack() as ctx:
        cpool = ctx.enter_context(tc.tile_pool(name="consts", bufs=1))
        wk_t = cpool.tile([124, 64], BF16)
        nc.sync.dma_start(out=wk_t[:], in_=wk_d[:])
        cb_t = cpool.tile([128, 1], F32)
        nc.sync.dma_start(out=cb_t[:], in_=cb_d[:])
        id_t = cpool.tile([128, 128], F16)
        nc.sync.dma_start(out=id_t[:], in_=id_d[:])

        gin = ctx.enter_context(tc.tile_pool(name="gin", bufs=2))
        gwork = ctx.enter_context(tc.tile_pool(name="gwork", bufs=3))
        arp = ctx.enter_context(tc.tile_pool(name="arp", bufs=2))
        kp = ctx.enter_context(tc.tile_pool(name="kp", bufs=6))
        mbp = ctx.enter_context(tc.tile_pool(name="mbp", bufs=3))
        mep = ctx.enter_context(tc.tile_pool(name="mep", bufs=2))
        outp = ctx.enter_context(tc.tile_pool(name="outp", bufs=2))
        ps_t = ctx.enter_context(tc.tile_pool(name="ps_t", bufs=4, space="PSUM"))
        ps_m = ctx.enter_context(tc.tile_pool(name="ps_m", bufs=4, space="PSUM"))

        def T32(tag):
            return gwork.tile([128, C], F32, tag=tag, name=tag)

        def T16(tag):
            return gwork.tile([128, C], BF16, tag=tag, name=tag)

        def Ts32(tag):
            return gwork.tile([128, C], F32, tag="scr32", name=tag, bufs=10)

        def Ts16(tag):
            return gwork.tile([128, C], BF16, tag="scr16", name=tag, bufs=14)

        def load(s):
            Lt = gin.tile([128, 9, C], F32, tag="Lt", name="Lt")
            Rt = gin.tile([128, 9, C], F32, tag="Rt", name="Rt")
            nc.sync.dma_start(out=Lt[:], in_=lig[s])
            nc.sync.dma_start(out=Rt[:], in_=rec[s])
            return Lt, Rt

        def stage_A(s, ld):
            st = {}
            Lt, Rt = ld
            ev = [T32(f"ev{r}") for r in range(9)]
            for r in range(9):
                nc.vector.tensor_tensor(ev[r][:], Lt[:, r, :], Rt[:, r, :], OP.subtract)
            sq = [Ts32(f"sq{r}") for r in range(9)]
            for r in range(9):
                nc.gpsimd.tensor_tensor(sq[r][:], ev[r][:], ev[r][:], OP.mult)
            d2 = []
            for j in range(3):
                t = Ts32(f"d2a{j}")
                nc.vector.tensor_tensor(t[:], sq[3 * j][:], sq[3 * j + 1][:], OP.add)
                dj = T32(f"d2{j}")
                nc.vector.tensor_tensor(dj[:], t[:], sq[3 * j + 2][:], OP.add)
                d2.append(dj)
            rin, dv = [], []
            for j in range(3):
                ln = Ts32(f"ln{j}")
                nc.scalar.activation(ln[:], d2[j][:], AF.Ln)
                ri = T16(f"ri{j}")
                nc.scalar.activation(ri[:], ln[:], AF.Exp, scale=-0.5)
                dj = T32(f"d{j}")
                nc.scalar.activation(dj[:], ln[:], AF.Exp, scale=0.5)
                rin.append(ri); dv.append(dj)
            u = [T16(f"u{r}") for r in range(9)]
            for j in range(3):
                for k in range(3):
                    nc.vector.tensor_tensor(u[3 * j + k][:], ev[3 * j + k][:],
                                            rin[j][:], OP.mult)
            st["u"] = u

            def dot(ja, jb, tag):
                m0, m1, m2 = Ts16(tag + "m0"), Ts16(tag + "m1"), Ts16(tag + "m2")
                nc.gpsimd.tensor_tensor(m0[:], u[3 * ja][:], u[3 * jb][:], OP.mult)
                nc.gpsimd.tensor_tensor(m1[:], u[3 * ja + 1][:], u[3 * jb + 1][:], OP.mult)
                nc.vector.tensor_tensor(m2[:], u[3 * ja + 2][:], u[3 * jb + 2][:], OP.mult)
                a = Ts16(tag + "a")
                nc.vector.tensor_tensor(a[:], m0[:], m1[:], OP.add)
                d_ = T16(tag)
                nc.vector.tensor_tensor(d_[:], a[:], m2[:], OP.add)
                return d_
            st["t12"] = dot(0, 1, "t12")
            st["t23"] = dot(1, 2, "t23")
            st["t13"] = dot(0, 2, "t13")

            def cross(ja, jb, tag):
                comp = []
                for k in range(3):
                    k1, k2 = (k + 1) % 3, (k + 2) % 3
                    a = Ts16(tag + f"a{k}")
                    nc.gpsimd.tensor_tensor(a[:], u[3 * ja + k1][:], u[3 * jb + k2][:], OP.mult)
                    b = Ts16(tag + f"b{k}")
                    nc.gpsimd.tensor_tensor(b[:], u[3 * ja + k2][:], u[3 * jb + k1][:], OP.mult)
                    c_ = T16(tag + f"{k}")
                    nc.vector.tensor_tensor(c_[:], a[:], b[:], OP.subtract)
                    comp.append(c_)
                return comp
            st["cr"] = cross(0, 1, "cr")
            st["c23"] = cross(1, 2, "c23")
            st["c31"] = cross(2, 0, "c31")
            cr = st["cr"]
            Dm0, Dm1 = Ts16("Dm0"), Ts16("Dm1")
            nc.gpsimd.tensor_tensor(Dm0[:], cr[0][:], u[6][:], OP.mult)
            nc.gpsimd.tensor_tensor(Dm1[:], cr[1][:], u[7][:], OP.mult)
            Dm2 = Ts16("Dm2")
            nc.vector.tensor_tensor(Dm2[:], cr[2][:], u[8][:], OP.mult)
            Da = Ts16("Da")
            nc.vector.tensor_tensor(Da[:], Dm0[:], Dm1[:], OP.add)
            Dt = T16("Dt")
            nc.vector.tensor_tensor(Dt[:], Da[:], Dm2[:], OP.add)
            st["Dt"] = Dt

            AR = arp.tile([128, 128, C], F16, tag="AR", name="AR")
            for j in range(3):
                for i in range(4):
                    nc.vector.scalar_tensor_tensor(
                        AR[:, 112 + 4 * j + i, :], dv[j][:], float(-2.0 * OFF[i]),
                        d2[j][:], OP.mult, OP.add)
            a1 = [AR[:, 112 + i, :] for i in range(4)]
            a2 = [AR[:, 116 + i, :] for i in range(4)]
            a3 = [AR[:, 120 + i, :] for i in range(4)]
            for p in range(4):
                for q in range(4):
                    nc.vector.tensor_tensor(AR[:, 64 + 4 * p + q, :], a1[p], a2[q], OP.add)
            for p in range(4):
                for q in range(4):
                    for v in range(4):
                        nc.vector.tensor_tensor(
                            AR[:, 16 * p + 4 * q + v, :], AR[:, 64 + 4 * p + q, :],
                            a3[v], OP.add)
            for p in range(4):
                for v in range(4):
                    nc.vector.tensor_tensor(AR[:, 80 + 4 * p + v, :], a1[p], a3[v], OP.add)
            for q in range(4):
                for v in range(4):
                    nc.vector.tensor_tensor(AR[:, 96 + 4 * q + v, :], a2[q], a3[v], OP.add)
            for i in range(4):
                nc.gpsimd.tensor_copy(AR[:, 124 + i, :], a1[i])
            st["AR"] = AR
            return st

        def stage_B(st):
            AR = st["AR"]
            ME = mep.tile([128, NB, PB, 64], BF16, tag="ME", name="ME")
            for bt in range(NB // TB):
                mb = mbp.tile([64, TB * PB * 128], BF16, tag="mb", name="mb")
                for bb in range(TB):
                    b = bt * TB + bb
                    pt = ps_t.tile([128, PB * 128], F16, tag="pt", name="pt")
                    for cc in range(PB):
                        c = b * PB + cc
                        nc.tensor.transpose(pt[:, cc * 128:(cc + 1) * 128],
                                            AR[:, :, c], id_t[:])
                    kt = kp.tile([124, PB * 128], BF16, tag="kt", name="kt")
                    nc.scalar.activation(kt[:], pt[0:124, :], AF.Exp,
                                         bias=cb_t[0:124, 0:1], scale=float(COEFF))
                    pm = ps_m.tile([64, PB * 128], F32, tag="pm", name="pm")
                    nc.tensor.matmul(pm[:], wk_t[:], kt[:], start=True, stop=True)
                    sl = mb[:, bb * PB * 128:(bb + 1) * PB * 128]
                    if bb % 4 == 0:
                        nc.scalar.copy(sl, pm[:])
                    else:
                        nc.vector.tensor_copy(sl, pm[:])
                nc.sync.dma_start_transpose(
                    ME[:, bt * TB:(bt + 1) * TB].rearrange("p b c m -> p (b c) m"),
                    mb[:])
            st["ME"] = ME

        def stage_C(st, s):
            u, cr, c23, c31 = st["u"], st["cr"], st["c23"], st["c31"]
            t12, t23, t13, Dt = st["t12"], st["t23"], st["t13"], st["Dt"]
            MEr = st["ME"][:].rearrange("p b c m -> p m (b c)")

            def me(m):
                return MEr[:, m, :]

            OT = outp.tile([128, 12, C], BF16, tag="OT", name="OT")
            ot = [OT[:, i, :] for i in range(12)]
            dots_for = [t12, t23, t13]
            for w in range(9):
                nc.vector.scalar_tensor_tensor(ot[w], Dt[:], float(G5[w]),
                                               me(w), OP.mult, OP.add)
                for tix, dtile in enumerate(dots_for):
                    tm = Ts16(f"as{w}_{tix}")
                    nc.gpsimd.tensor_tensor(tm[:], dtile[:], me(9 * (tix + 1) + w), OP.mult)
                    nc.vector.tensor_tensor(ot[w], ot[w], tm[:], OP.add)
            cu3, cu2, cu1 = T16("cu3"), T16("cu2"), T16("cu1")
            nc.vector.scalar_tensor_tensor(cu3[:], t12[:], h2p, me(36), OP.mult, OP.add)
            nc.vector.scalar_tensor_tensor(cu2[:], t13[:], kap, me(37), OP.mult, OP.add)
            nc.vector.scalar_tensor_tensor(cu1[:], t23[:], -kap, me(38), OP.mult, OP.add)
            for k in range(3):
                acc = ot[9 + k]
                nc.vector.tensor_tensor(acc, cu3[:], u[6 + k][:], OP.mult)
                for vi, (coef, vec) in enumerate([
                        (cu2[:], u[3 + k][:]), (cu1[:], u[k][:]),
                        (me(39), cr[k][:]), (me(40), c23[k][:]),
                        (me(41), c31[k][:])]):
                    t_ = Ts16(f"av{k}_{vi}")
                    nc.gpsimd.tensor_tensor(t_[:], coef, vec, OP.mult)
                    nc.vector.tensor_tensor(acc, acc, t_[:], OP.add)
            nc.sync.dma_start(out=out[s], in_=OT[:])

        from contextlib import nullcontext
        loop_cm = tc.For_i(0, reps, 1) if reps > 1 else nullcontext()
        with loop_cm:
            # Emission order per iteration: prefetch-DMA first, then the
            # stages whose inputs are already resident (B, C), then the fresh
            # compute (A) whose ops stall on the newest DMA/cross-engine
            # results.  Ready work sits ahead of stalling work in every
            # engine's program order, avoiding head-of-line blocking in the
            # shallow per-engine wait queues.
            sts, lds = {}, {}
            for s in range(S + 2):
                if s == 0:
                    lds[0] = load(0)
                if s + 1 < S:
                    lds[s + 1] = load(s + 1)
                if 1 <= s <= S:
                    stage_B(sts[s - 1])
                if s >= 2:
                    stage_C(sts[s - 2], s - 2)
                    del sts[s - 2]
                if s < S:
                    sts[s] = stage_A(s, lds.pop(s))

    nc.compile()
    return nc


def prep_inputs(ligand, receptor, n_cores, S, C):
    B = ligand.shape[0]
    Bp = S * 128 * C * n_cores

    def conv(x, padval):
        xr = np.ascontiguousarray(
            np.asarray(x, np.float32)[:, 1:4, :].reshape(B, 9).T)
        if Bp > B:
            pad = np.full((9, Bp - B), padval, np.float32)
            xr = np.concatenate([xr, pad], axis=1)
        xr = xr.reshape(9, n_cores, S, C, 128).transpose(1, 2, 4, 0, 3)
        return np.ascontiguousarray(xr)
    # pad: lig 1.0 / rec 0.0 -> unit-ish edge vectors, finite math
    return conv(ligand, 1.0), conv(receptor, 0.0)


def unpack_output(outs, B, n_cores, S, C):
    full = np.stack(outs).astype(np.float32)   # (ncore, S, 128, 12, C)
    full = full.transpose(3, 0, 1, 4, 2).reshape(12, -1)[:, :B]
    o = np.ascontiguousarray(full.T)
    return o[:, 0:3], o[:, 3:6], o[:, 6:9], o[:, 9:12]


_CACHE = {}


def _get_nc(S, C, weights):
    key = (S, C) + tuple(np.asarray(v).tobytes() for v in weights.values())
    h = hash(key)
    if h not in _CACHE:
        _CACHE[h] = build_nc(S, C, weights, N_CORES)
    return _CACHE[h]


def kernel(ligand, receptor, w1_000, w1_110, w1_011, w1_101, w1_111,
           w2_000, w2_110, w2_011, w2_101, w2_111):
    B = int(np.asarray(ligand).shape[0])
    C = C_CHUNK
    S = max(1, -(-B // (128 * C * N_CORES)))
    weights = dict(w1_000=w1_000, w1_110=w1_110, w1_011=w1_011,
                   w1_101=w1_101, w1_111=w1_111, w2_000=w2_000,
                   w2_110=w2_110, w2_011=w2_011, w2_101=w2_101,
                   w2_111=w2_111)
    nc = _get_nc(S, C, weights)
    ligs, recs = prep_inputs(ligand, receptor, N_CORES, S, C)
    in_maps = [{"lig": ligs[i], "rec": recs[i]} for i in range(N_CORES)]
    res = run_bass_kernel_spmd(nc, in_maps, list(range(N_CORES)))
    outs = [res.results[i]["out"] for i in range(N_CORES)]
    return unpack_output(outs, B, N_CORES, S, C)



# revision 43
# speedup vs baseline: 1.1016x; 1.1016x over previous
"""FrameDockingScoreModel forward as a Trainium2 Bass kernel (8 NeuronCores).

Contract: kernel(**inputs) takes the FULL unsharded inputs from
setup_inputs() and returns (s_rot, s_tr, t_rot, t_tr), float32 (B, 3) each.

Math: the two chained e3nn tensor products are linearized at build time into
one K=124 -> M=42 matmul over per-edge exponential features:
  s1p*s2q*s3v = exp(coeff*(a1[p]+a2[q]+a3[v]) + bias_pqv),  a_j[i] = d2_j - 2*off_i*d_j
so the whole Gaussian-product feature block is ONE activation over a
transposed argument tile.  Per 16384-edge sub-group, per core:
  A: geometry (edge-major SoA tiles, DVE/Pool/ACT) -> argument tile AR (fp16)
  B: PE-transpose AR chunks -> PSUM, ACT exp (PSUM->SBUF bf16 K-tile),
     bf16 matmul vs folded weight table, escape-cast, HW DMA-transpose back
  C: edge-major assembly of the 12 outputs (bf16) -> DMA out
Stages are software-pipelined (A(s), B(s-1), C(s-2) emission skew) so each
engine's program order interleaves independent sub-groups.
Batch is padded to 8 * S * 128 * C edges and split across the 8 cores.
"""
import numpy as np
import ml_dtypes
from contextlib import ExitStack

import concourse.bacc as bacc
import concourse.tile as tile
from concourse import mybir
from concourse.bass_utils import run_bass_kernel_spmd

NG = 4
MAX_OFF = 5.0
C1_000 = (1.0 / 17.0) ** 0.5
C1_110 = (1.0 / 17.0) ** 0.5 / 3.0 ** 0.5
C1_011 = (3.0 / 9.0) ** 0.5 / 3.0 ** 0.5
C1_101 = C1_011
C1_111 = (3.0 / 9.0) ** 0.5 / 6.0 ** 0.5
C2_000 = (1.0 / 85.0) ** 0.5
C2_110 = (1.0 / 85.0) ** 0.5 / 3.0 ** 0.5
C2_011 = (3.0 / 45.0) ** 0.5 / 3.0 ** 0.5
C2_101 = C2_011
C2_111 = (3.0 / 45.0) ** 0.5 / 6.0 ** 0.5

F32 = mybir.dt.float32
F16 = mybir.dt.float16
BF16 = mybir.dt.bfloat16
AF = mybir.ActivationFunctionType
OP = mybir.AluOpType

OFF = np.linspace(0.0, MAX_OFF, NG).astype(np.float64)
COEFF = -0.5 / (OFF[1] - OFF[0]) ** 2

N_CORES = 8
C_CHUNK = 128            # edges-per-partition per sub-group


def build_weight_tables(w):
    w = {k: np.asarray(v, np.float64) for k, v in w.items()}
    G1 = C2_000 * C1_000 * np.einsum('uvw,pqu->pqvw', w['w2_000'], w['w1_000'])
    G2 = C2_000 * C1_110 * np.einsum('uvw,u->vw', w['w2_000'], w['w1_110'])
    G3 = C2_110 * C1_011 * np.einsum('uw,pu->pw', w['w2_110'], w['w1_011'])
    G4 = C2_110 * C1_101 * np.einsum('uw,qu->qw', w['w2_110'], w['w1_101'])
    G5 = C2_110 * C1_111 * np.einsum('uw,u->w', w['w2_110'], w['w1_111'])
    H1p = C2_011 * C1_000 * np.einsum('u,pqu->pq', w['w2_011'], w['w1_000'])
    h2p = C2_011 * C1_110 * float(w['w2_011'] @ w['w1_110'])
    H3 = C2_101 * C1_011 * np.einsum('uv,pu->pv', w['w2_101'], w['w1_011'])
    H4 = C2_101 * C1_101 * np.einsum('uv,qu->qv', w['w2_101'], w['w1_101'])
    H5 = C2_101 * C1_111 * np.einsum('uv,u->v', w['w2_101'], w['w1_111'])
    E1 = C2_111 * C1_011 * np.einsum('u,pu->p', w['w2_111'], w['w1_011'])
    E2 = C2_111 * C1_101 * np.einsum('u,qu->q', w['w2_111'], w['w1_101'])
    kap = C2_111 * C1_111 * float(w['w2_111'] @ w['w1_111'])

    Wk = np.zeros((124, 64), np.float64)
    r = 0
    Wk[r:r + 64, 0:9] = G1.reshape(64, 9); r += 64       # s123 -> S2p
    Wk[r:r + 16, 36] = H1p.reshape(16); r += 16          # s12 -> alpha
    Wk[r:r + 16, 37] = H3.reshape(16); r += 16           # s13 -> beta
    Wk[r:r + 16, 38] = H4.reshape(16); r += 16           # s23 -> gamma
    Wk[r:r + 4, 18:27] = G3; Wk[r:r + 4, 40] = E1; r += 4    # s1 -> T3, z1
    Wk[r:r + 4, 27:36] = G4; Wk[r:r + 4, 41] = -E2; r += 4   # s2 -> T4, z2m
    Wk[r:r + 4, 9:18] = G2; Wk[r:r + 4, 39] = H5; r += 4     # s3 -> T2, delta'
    assert r == 124

    o2 = OFF ** 2
    p_, q_, v_ = np.meshgrid(np.arange(4), np.arange(4), np.arange(4), indexing='ij')
    pq = np.meshgrid(np.arange(4), np.arange(4), indexing='ij')
    cb = np.concatenate([
        o2[p_.ravel()] + o2[q_.ravel()] + o2[v_.ravel()],
        o2[pq[0].ravel()] + o2[pq[1].ravel()],
        o2[pq[0].ravel()] + o2[pq[1].ravel()],
        o2[pq[0].ravel()] + o2[pq[1].ravel()],
        o2, o2, o2,
        np.zeros(4),
    ]) * COEFF
    cbias = cb.reshape(128, 1).astype(np.float32)
    return (Wk.astype(np.float32), cbias, G5.astype(np.float32),
            float(h2p), float(kap))


def build_nc(S, C, weights, n_cores, reps=1, TB=4, PB=8):
    Wk64, cbias, G5, h2p, kap = build_weight_tables(weights)
    NB = C // PB
    assert NB % TB == 0

    nc = bacc.Bacc("TRN2", target_bir_lowering=False, debug=False,
                   num_devices=n_cores)
    lig = nc.declare_dram_parameter("lig", [S, 128, 9, C], F32, isOutput=False)
    rec = nc.declare_dram_parameter("rec", [S, 128, 9, C], F32, isOutput=False)
    out = nc.declare_dram_parameter("out", [S, 128, 12, C], BF16, isOutput=True)

    wk_d = nc.inline_tensor(Wk64.astype(ml_dtypes.bfloat16), name="wk")
    cb_d = nc.inline_tensor(cbias, name="cbias")
    id_d = nc.inline_tensor(np.eye(128, dtype=np.float16), name="ident16")

    with tile.TileContext(nc) as tc, ExitStack() as ctx:
        cpool = ctx.enter_context(tc.tile_pool(name="consts", bufs=1))
        wk_t = cpool.tile([124, 64], BF16)
        nc.sync.dma_start(out=wk_t[:], in_=wk_d[:])
        cb_t = cpool.tile([128, 1], F32)
        nc.sync.dma_start(out=cb_t[:], in_=cb_d[:])
        id_t = cpool.tile([128, 128], F16)
        nc.sync.dma_start(out=id_t[:], in_=id_d[:])

        gin = ctx.enter_context(tc.tile_pool(name="gin", bufs=2))
        gwork = ctx.enter_context(tc.tile_pool(name="gwork", bufs=3))
        arp = ctx.enter_context(tc.tile_pool(name="arp", bufs=2))
        kp = ctx.enter_context(tc.tile_pool(name="kp", bufs=3))
        mbp = ctx.enter_context(tc.tile_pool(name="mbp", bufs=3))
        mep = ctx.enter_context(tc.tile_pool(name="mep", bufs=2))
        outp = ctx.enter_context(tc.tile_pool(name="outp", bufs=2))
        ps_t = ctx.enter_context(tc.tile_pool(name="ps_t", bufs=3, space="PSUM"))
        ps_m = ctx.enter_context(tc.tile_pool(name="ps_m", bufs=4, space="PSUM"))

        def T32(tag):
            return gwork.tile([128, C], F32, tag=tag, name=tag)

        def T16(tag):
            return gwork.tile([128, C], BF16, tag=tag, name=tag)

        def Ts32(tag):
            return gwork.tile([128, C], F32, tag="scr32", name=tag, bufs=10)

        def Ts16(tag):
            return gwork.tile([128, C], BF16, tag="scr16", name=tag, bufs=14)

        def load(s):
            Lt = gin.tile([128, 9, C], F32, tag="Lt", name="Lt")
            Rt = gin.tile([128, 9, C], F32, tag="Rt", name="Rt")
            nc.sync.dma_start(out=Lt[:], in_=lig[s])
            nc.sync.dma_start(out=Rt[:], in_=rec[s])
            return Lt, Rt

        def stage_A(s, ld):
            st = {}
            Lt, Rt = ld
            ev = [T32(f"ev{r}") for r in range(9)]
            for r in range(9):
                nc.any.tensor_tensor(ev[r][:], Lt[:, r, :], Rt[:, r, :], OP.subtract)
            sq = [Ts32(f"sq{r}") for r in range(9)]
            for r in range(9):
                nc.any.tensor_tensor(sq[r][:], ev[r][:], ev[r][:], OP.mult)
            d2 = []
            for j in range(3):
                t = Ts32(f"d2a{j}")
                nc.any.tensor_tensor(t[:], sq[3 * j][:], sq[3 * j + 1][:], OP.add)
                dj = T32(f"d2{j}")
                nc.any.tensor_tensor(dj[:], t[:], sq[3 * j + 2][:], OP.add)
                d2.append(dj)
            rin, dv = [], []
            for j in range(3):
                ln = Ts32(f"ln{j}")
                nc.scalar.activation(ln[:], d2[j][:], AF.Ln)
                ri = T16(f"ri{j}")
                nc.scalar.activation(ri[:], ln[:], AF.Exp, scale=-0.5)
                dj = T32(f"d{j}")
                nc.scalar.activation(dj[:], ln[:], AF.Exp, scale=0.5)
                rin.append(ri); dv.append(dj)
            u = [T16(f"u{r}") for r in range(9)]
            for j in range(3):
                for k in range(3):
                    nc.any.tensor_tensor(u[3 * j + k][:], ev[3 * j + k][:],
                                            rin[j][:], OP.mult)
            st["u"] = u

            def dot(ja, jb, tag):
                m0, m1, m2 = Ts16(tag + "m0"), Ts16(tag + "m1"), Ts16(tag + "m2")
                nc.any.tensor_tensor(m0[:], u[3 * ja][:], u[3 * jb][:], OP.mult)
                nc.any.tensor_tensor(m1[:], u[3 * ja + 1][:], u[3 * jb + 1][:], OP.mult)
                nc.any.tensor_tensor(m2[:], u[3 * ja + 2][:], u[3 * jb + 2][:], OP.mult)
                a = Ts16(tag + "a")
                nc.any.tensor_tensor(a[:], m0[:], m1[:], OP.add)
                d_ = T16(tag)
                nc.any.tensor_tensor(d_[:], a[:], m2[:], OP.add)
                return d_
            st["t12"] = dot(0, 1, "t12")
            st["t23"] = dot(1, 2, "t23")
            st["t13"] = dot(0, 2, "t13")

            def cross(ja, jb, tag):
                comp = []
                for k in range(3):
                    k1, k2 = (k + 1) % 3, (k + 2) % 3
                    a = Ts16(tag + f"a{k}")
                    nc.any.tensor_tensor(a[:], u[3 * ja + k1][:], u[3 * jb + k2][:], OP.mult)
                    b = Ts16(tag + f"b{k}")
                    nc.any.tensor_tensor(b[:], u[3 * ja + k2][:], u[3 * jb + k1][:], OP.mult)
                    c_ = T16(tag + f"{k}")
                    nc.any.tensor_tensor(c_[:], a[:], b[:], OP.subtract)
                    comp.append(c_)
                return comp
            st["cr"] = cross(0, 1, "cr")
            st["c23"] = cross(1, 2, "c23")
            st["c31"] = cross(2, 0, "c31")
            cr = st["cr"]
            Dm0, Dm1 = Ts16("Dm0"), Ts16("Dm1")
            nc.any.tensor_tensor(Dm0[:], cr[0][:], u[6][:], OP.mult)
            nc.any.tensor_tensor(Dm1[:], cr[1][:], u[7][:], OP.mult)
            Dm2 = Ts16("Dm2")
            nc.any.tensor_tensor(Dm2[:], cr[2][:], u[8][:], OP.mult)
            Da = Ts16("Da")
            nc.any.tensor_tensor(Da[:], Dm0[:], Dm1[:], OP.add)
            Dt = T16("Dt")
            nc.any.tensor_tensor(Dt[:], Da[:], Dm2[:], OP.add)
            st["Dt"] = Dt

            AR = arp.tile([128, 128, C], F16, tag="AR", name="AR")
            for j in range(3):
                for i in range(4):
                    nc.vector.scalar_tensor_tensor(
                        AR[:, 112 + 4 * j + i, :], dv[j][:], float(-2.0 * OFF[i]),
                        d2[j][:], OP.mult, OP.add)
            a1 = [AR[:, 112 + i, :] for i in range(4)]
            a2 = [AR[:, 116 + i, :] for i in range(4)]
            a3 = [AR[:, 120 + i, :] for i in range(4)]
            for p in range(4):
                for q in range(4):
                    nc.any.tensor_tensor(AR[:, 64 + 4 * p + q, :], a1[p], a2[q], OP.add)
            for p in range(4):
                for q in range(4):
                    for v in range(4):
                        nc.any.tensor_tensor(
                            AR[:, 16 * p + 4 * q + v, :], AR[:, 64 + 4 * p + q, :],
                            a3[v], OP.add)
            for p in range(4):
                for v in range(4):
                    nc.any.tensor_tensor(AR[:, 80 + 4 * p + v, :], a1[p], a3[v], OP.add)
            for q in range(4):
                for v in range(4):
                    nc.any.tensor_tensor(AR[:, 96 + 4 * q + v, :], a2[q], a3[v], OP.add)
            for i in range(4):
                nc.any.tensor_copy(AR[:, 124 + i, :], a1[i])
            st["AR"] = AR
            return st

        def stage_B(st):
            AR = st["AR"]
            ME = mep.tile([128, NB, PB, 64], BF16, tag="ME", name="ME")
            for bt in range(NB // TB):
                mb = mbp.tile([64, TB * PB * 128], BF16, tag="mb", name="mb")
                for bb in range(TB):
                    b = bt * TB + bb
                    pt = ps_t.tile([128, PB * 128], F16, tag="pt", name="pt")
                    for cc in range(PB):
                        c = b * PB + cc
                        nc.tensor.transpose(pt[:, cc * 128:(cc + 1) * 128],
                                            AR[:, :, c], id_t[:])
                    kt = kp.tile([124, PB * 128], BF16, tag="kt", name="kt")
                    nc.scalar.activation(kt[:], pt[0:124, :], AF.Exp,
                                         bias=cb_t[0:124, 0:1], scale=float(COEFF))
                    # PSUM bank limit: N<=512 f32 per matmul output
                    for half in range(PB * 128 // 512):
                        pm = ps_m.tile([64, 512], F32, tag="pm", name="pm")
                        nc.tensor.matmul(pm[:], wk_t[:],
                                         kt[:, half * 512:(half + 1) * 512],
                                         start=True, stop=True)
                        sl = mb[:, bb * PB * 128 + half * 512:
                                bb * PB * 128 + (half + 1) * 512]
                        nc.any.tensor_copy(sl, pm[:])
                nc.sync.dma_start_transpose(
                    ME[:, bt * TB:(bt + 1) * TB].rearrange("p b c m -> p (b c) m"),
                    mb[:])
            st["ME"] = ME

        def stage_C(st, s):
            u, cr, c23, c31 = st["u"], st["cr"], st["c23"], st["c31"]
            t12, t23, t13, Dt = st["t12"], st["t23"], st["t13"], st["Dt"]
            MEr = st["ME"][:].rearrange("p b c m -> p m (b c)")

            def me(m):
                return MEr[:, m, :]

            OT = outp.tile([128, 12, C], BF16, tag="OT", name="OT")
            ot = [OT[:, i, :] for i in range(12)]
            dots_for = [t12, t23, t13]
            for w in range(9):
                nc.vector.scalar_tensor_tensor(ot[w], Dt[:], float(G5[w]),
                                               me(w), OP.mult, OP.add)
                for tix, dtile in enumerate(dots_for):
                    tm = Ts16(f"as{w}_{tix}")
                    nc.any.tensor_tensor(tm[:], dtile[:], me(9 * (tix + 1) + w), OP.mult)
                    nc.any.tensor_tensor(ot[w], ot[w], tm[:], OP.add)
            cu3, cu2, cu1 = T16("cu3"), T16("cu2"), T16("cu1")
            nc.vector.scalar_tensor_tensor(cu3[:], t12[:], h2p, me(36), OP.mult, OP.add)
            nc.vector.scalar_tensor_tensor(cu2[:], t13[:], kap, me(37), OP.mult, OP.add)
            nc.vector.scalar_tensor_tensor(cu1[:], t23[:], -kap, me(38), OP.mult, OP.add)
            for k in range(3):
                acc = ot[9 + k]
                nc.any.tensor_tensor(acc, cu3[:], u[6 + k][:], OP.mult)
                for vi, (coef, vec) in enumerate([
                        (cu2[:], u[3 + k][:]), (cu1[:], u[k][:]),
                        (me(39), cr[k][:]), (me(40), c23[k][:]),
                        (me(41), c31[k][:])]):
                    t_ = Ts16(f"av{k}_{vi}")
                    nc.any.tensor_tensor(t_[:], coef, vec, OP.mult)
                    nc.any.tensor_tensor(acc, acc, t_[:], OP.add)
            nc.sync.dma_start(out=out[s], in_=OT[:])

        from contextlib import nullcontext
        loop_cm = tc.For_i(0, reps, 1) if reps > 1 else nullcontext()
        with loop_cm:
            # Emission order per iteration: prefetch-DMA first, then the
            # stages whose inputs are already resident (B, C), then the fresh
            # compute (A) whose ops stall on the newest DMA/cross-engine
            # results.  Ready work sits ahead of stalling work in every
            # engine's program order, avoiding head-of-line blocking in the
            # shallow per-engine wait queues.
            sts, lds = {}, {}
            for s in range(S + 2):
                if s == 0:
                    lds[0] = load(0)
                if s + 1 < S:
                    lds[s + 1] = load(s + 1)
                if 1 <= s <= S:
                    stage_B(sts[s - 1])
                if s >= 2:
                    stage_C(sts[s - 2], s - 2)
                    del sts[s - 2]
                if s < S:
                    sts[s] = stage_A(s, lds.pop(s))

    nc.compile()
    return nc


def prep_inputs(ligand, receptor, n_cores, S, C):
    B = ligand.shape[0]
    Bp = S * 128 * C * n_cores

    def conv(x, padval):
        xr = np.ascontiguousarray(
            np.asarray(x, np.float32)[:, 1:4, :].reshape(B, 9).T)
        if Bp > B:
            pad = np.full((9, Bp - B), padval, np.float32)
            xr = np.concatenate([xr, pad], axis=1)
        xr = xr.reshape(9, n_cores, S, C, 128).transpose(1, 2, 4, 0, 3)
        return np.ascontiguousarray(xr)
    # pad: lig 1.0 / rec 0.0 -> unit-ish edge vectors, finite math
    return conv(ligand, 1.0), conv(receptor, 0.0)


def unpack_output(outs, B, n_cores, S, C):
    full = np.stack(outs).astype(np.float32)   # (ncore, S, 128, 12, C)
    full = full.transpose(3, 0, 1, 4, 2).reshape(12, -1)[:, :B]
    o = np.ascontiguousarray(full.T)
    return o[:, 0:3], o[:, 3:6], o[:, 6:9], o[:, 9:12]


_CACHE = {}


def _get_nc(S, C, weights):
    key = (S, C) + tuple(np.asarray(v).tobytes() for v in weights.values())
    h = hash(key)
    if h not in _CACHE:
        _CACHE[h] = build_nc(S, C, weights, N_CORES)
    return _CACHE[h]


def kernel(ligand, receptor, w1_000, w1_110, w1_011, w1_101, w1_111,
           w2_000, w2_110, w2_011, w2_101, w2_111):
    B = int(np.asarray(ligand).shape[0])
    C = C_CHUNK
    S = max(1, -(-B // (128 * C * N_CORES)))
    weights = dict(w1_000=w1_000, w1_110=w1_110, w1_011=w1_011,
                   w1_101=w1_101, w1_111=w1_111, w2_000=w2_000,
                   w2_110=w2_110, w2_011=w2_011, w2_101=w2_101,
                   w2_111=w2_111)
    nc = _get_nc(S, C, weights)
    ligs, recs = prep_inputs(ligand, receptor, N_CORES, S, C)
    in_maps = [{"lig": ligs[i], "rec": recs[i]} for i in range(N_CORES)]
    res = run_bass_kernel_spmd(nc, in_maps, list(range(N_CORES)))
    outs = [res.results[i]["out"] for i in range(N_CORES)]
    return unpack_output(outs, B, N_CORES, S, C)

